# revision 23
# baseline (speedup 1.0000x reference)
"""BertAttention Trainium2 kernel (8 NeuronCores, SPMD).

Sharding: core c handles batch b = c//2 and head-half hh = c%2 (8 of 16 heads).
Each core computes q/k/v projections for its 512 head-dims over its batch's
full sequence, per-head attention (no mask, scale 1/sqrt(1024)), and a partial
o-projection over its 512 context dims.  Host sums the two partials per batch.

v2 design (ACT-exp is the hard floor at ~285us/core; PE work reduced below it):
  QK  : unchanged (K=64 row-packed pairs, fp16, half-rate -- irreducible).
  exp : ACT over st [128,2,512] psum -> pt [128,2,512] f16, 256 instrs.
  AV  : SWAPPED dataflow -- stationary = P^T chunk [128k,128q] (from pt),
        moving = V_aug [128k,65] (64 v-dims + ones col). out = ctx [128q,65]
        psum, accumulated over 16 k-tiles per (head j, q-chunk qc).  Full
        128-wide array => ~2x fewer PE cycles than the ctx^T form.  The
        softmax denominator lands in psum COLUMN 64 -> per-partition DVE
        reciprocal + tensor_scalar_mul (no broadcast matmuls, no [64,2048]
        DVE multiplies).
  ctxT: PE transpose (identity matmul) of normalized ctx [128q,64] ->
        [64,128] written at column-position 64j, gpsimd copies psum->sbuf.
  oproj: as baseline (per-pair psum chunks, DVE add into out_sb), fp16 out.

Emission order keeps ACT continuously fed: per (pair p, qt) block, the 16
kt-slots emit QK+exp, and between them: AV chains of the PREVIOUS block
(1 chain per 2 slots), q/k projection bursts for pair p+1, v-projection
half-passes, and o-proj chunks for pair p-1.

PSUM (16KB/partition): st 2x[128,2,512]f32 (8KB) + acc 2x[128,65]f32 padded
to 2KB (4KB) + pp 2x2KB (proj/oproj/transpose, 4KB).
"""

import sys

sys.path.insert(0, "/opt/trn_rl_repo")

import numpy as np

B, S, D, H = 4, 2048, 1024, 16
HEAD = 64
NCORES = 8
P = 128
NQ = 512            # q free-tile width
KT_TILES = S // P   # 16 k tiles
QT_TILES = S // NQ  # 4 q tiles
DC = 8              # contraction chunks for projections (1024/128)
PAIRS = 4           # head pairs per core


_NC_CACHE = None


def _build_nc():
    import concourse.bass as bass  # noqa: F401
    import concourse.tile as tile
    from concourse import bacc, mybir

    f32 = mybir.dt.float32
    f16 = mybir.dt.float16
    AF = mybir.ActivationFunctionType

    nc = bacc.Bacc(None)
    xt_d = nc.declare_dram_parameter("xt", [DC, P, S], f16, isOutput=False)
    # per-pair contiguous weight images [P, DC*P]: one big DMA each (the
    # [DC,P,P] chunk layout produced 256B/partition packets, ~45us to land)
    wqt_d = nc.declare_dram_parameter("wqt", [PAIRS, P, DC * P], f16, isOutput=False)
    wkt_d = nc.declare_dram_parameter("wkt", [PAIRS, P, DC * P], f16, isOutput=False)
    wvt_d = nc.declare_dram_parameter("wvt", [DC, P, 512], f16, isOutput=False)
    woth_d = nc.declare_dram_parameter("woth", [PAIRS, P, D], f16, isOutput=False)
    ident_d = nc.declare_dram_parameter("ident", [P, P], f16, isOutput=False)
    out_d = nc.declare_dram_parameter("outt", [D // P, P, S], f16, isOutput=True)

    from contextlib import ExitStack

    with tile.TileContext(nc) as tc, ExitStack() as es:
        def pool(name, bufs, space="SBUF"):
            return es.enter_context(
                tc.tile_pool(name=name, bufs=bufs, space=space))

        xt_pool = pool("xt", 1)
        # 4 slots = all pairs' weights resident: pair p+1's weight DMA must
        # not wait on pair p's LAST (late-dribbled) proj burst, since pair
        # p+1's own bursts sit earlier in the in-order PE stream.
        wq_pool = pool("wq", 2)
        wk_pool = pool("wk", 2)
        wv_pool = pool("wv", 8)
        qt_pool = pool("qt", 2)
        kt_pool = pool("kt", 2)
        v_pool = pool("v", 1)
        pt_pool = pool("pt", 32)
        cn_pool = pool("cn", 4)
        rc_pool = pool("rc", 4)
        ctx_pool = pool("ctx", 1)
        wot_pool = pool("wot", 1)
        ost_pool = pool("ost", 1)
        on_pool = pool("on", 1)
        pp_pool = pool("pp", 2, "PSUM")
        st_pool = pool("st", 2, "PSUM")
        acc_pool = pool("acc", 2, "PSUM")

        # PE warmup during the initial DMA: keeps HAM at 8/8 so the
        # first projection matmuls run at 2.4 GHz
        wup = on_pool.tile([P, NQ], f16, tag="wup", name="wup")
        nc.vector.memset(wup[:], 0.125)
        wups = pp_pool.tile([P, NQ], f32, tag="pp", name="wups")
        for _ in range(42):
            nc.tensor.matmul(wups[:], wup[:, 0:P], wup[:],
                             start=True, stop=True)

        # load x^T chunks -- spread across 4 engine queues so the first
        # projections (which contract over ALL chunks) start ~4x sooner
        xt = []
        dma_engs = [nc.sync, nc.gpsimd, nc.scalar]
        for k in range(DC):
            t = xt_pool.tile([P, S], f16, tag=f"xt{k}", name=f"xt{k}")
            dma_engs[k % 3].dma_start(t[:], xt_d[k])
            xt.append(t)

        wv_t = []
        for k in range(DC):
            t = wv_pool.tile([P, NQ], f16, tag="wv", name="wv")
            dma_engs[(k + 1) % 3].dma_start(t[:], wvt_d[k])
            wv_t.append(t)

        # identity for PE transposes (after xt/wv in the queue)
        ident = on_pool.tile([P, P], f16, tag="id", name="ident")
        nc.gpsimd.dma_start(ident[:], ident_d[:, :])

        # V_aug tiles per k-tile: [128 keys, 4 heads, 65] (64 v-dims + ones)
        v_half = {0: [None] * KT_TILES, 1: [None] * KT_TILES}

        def proj_v(mt, half):
            """one N=256 projection pass filling v_half[half][mt]"""
            ps = pp_pool.tile([P, 256], f32, tag="pp", name="ppv")
            for k in range(DC):
                nc.tensor.matmul(
                    ps[:], xt[k][:, mt * P:(mt + 1) * P],
                    wv_t[k][:, half * 256:(half + 1) * 256],
                    start=(k == 0), stop=(k == DC - 1),
                )
            t = v_pool.tile([P, 4, 65], f16, tag=f"v{half}_{mt}",
                            name=f"v{half}_{mt}")
            nc.vector.memset(t[:], 1.0)
            v_half[half][mt] = t
            src = ps[:, :].rearrange("p (h d) -> p h d", h=4)
            nc.vector.tensor_copy(t[:, :, 0:64], src)

        def load_w(w_pool, w_dram, p):
            t = w_pool.tile([P, DC * P], f16, tag="w", name="w")
            nc.sync.dma_start(t[:], w_dram[p])
            return [t[:, k * P:(k + 1) * P] for k in range(DC)]

        def proj_nt(out, w_t, nt):
            ps = pp_pool.tile([P, NQ], f32, tag="pp", name="pp")
            for k in range(DC):
                nc.tensor.matmul(
                    ps[:], w_t[k][:], xt[k][:, nt * NQ:(nt + 1) * NQ],
                    start=(k == 0), stop=(k == DC - 1),
                )
            nc.vector.tensor_copy(out[:, nt * NQ:(nt + 1) * NQ], ps[:])

        # ---- per-pair state ----
        QTs, KTs, ctxTs = {}, {}, {}
        wot_t = {}
        wqk_t = {}
        out_sb = []
        for _ot in range(D // P):
            _t = ost_pool.tile([P, S], f16, tag=f"ou{_ot}", name=f"ou{_ot}")
            out_sb.append(_t)

        def load_wot(p):
            th = wot_pool.tile([P, D], f16, tag=f"woth{p}", name=f"woth{p}")
            nc.gpsimd.dma_start(th[:], woth_d[p])
            wot_t[p] = th

        def alloc_pair(p):
            KTs[p] = kt_pool.tile([P, S], f16, tag="t", name=f"kt{p}")
            QTs[p] = qt_pool.tile([P, S], f16, tag="t", name=f"qt{p}")
            ctxTs[p] = ctx_pool.tile([P, S], f16, tag=f"ctx{p}",
                                     name=f"ctx{p}")

        def oproj_chunk(p, qt, ots=None):
            for ot in (range(D // P) if ots is None else ots):
                ps = pp_pool.tile([P, NQ], f32, tag="pp", name="pp")
                nc.tensor.matmul(
                    ps[:], wot_t[p][:, ot * P:(ot + 1) * P],
                    ctxTs[p][:, qt * NQ:(qt + 1) * NQ],
                    start=True, stop=True,
                )
                dst = out_sb[ot][:, qt * NQ:(qt + 1) * NQ]
                if p == 0:
                    nc.vector.tensor_copy(dst, ps[:])
                else:
                    nc.vector.tensor_add(dst, dst, ps[:])
                if p == PAIRS - 1:
                    eng = nc.sync if ot % 2 == 0 else nc.gpsimd
                    eng.dma_start(out_d[ot][:, qt * NQ:(qt + 1) * NQ], dst)

        # pt tiles of the two most recent blocks
        pt_map = {}

        tp_box = [None]
        # deferred transpose: (bp, bqt, c, ctxn) emitted one chain later so
        # the PE never stalls on the DVE normalize (measured ~1.1us/chain)
        pend_tr = [None]

        def flush_transpose():
            if pend_tr[0] is None:
                return
            bp, bqt, c, ctxn = pend_tr[0]
            pend_tr[0] = None
            j, qc = c % 2, c // 2
            if j == 0:
                tp_box[0] = pp_pool.tile([P, P], f16, tag="pp", name="tp")
            tp = tp_box[0]
            nc.tensor.matmul(tp[64 * j:64 * (j + 1), :], ctxn[:], ident[:],
                             is_transpose=True)
            if j == 1:
                # gpsimd cannot read PSUM; DVE does the psum->sbuf hop
                nc.vector.tensor_copy(
                    ctxTs[bp][:, bqt * NQ + qc * P: bqt * NQ + (qc + 1) * P],
                    tp[:],
                )

        def av_chain(bp, bqt, c):
            """AV chain c (j = c%2, qc = c//2) of block (bp, bqt):
            ctx[q,d] accumulation + normalize; transpose deferred."""
            j, qc = c % 2, c // 2
            half = bp // 2
            jj = (2 * bp + j) % 4
            acc = acc_pool.tile([P, 65], f32, tag="acc", name="acc",
                                padded_shape=[P, 512])
            for i in range(KT_TILES):
                nc.tensor.matmul(
                    acc[:],
                    pt_map[(bp, bqt, i)][:, j, qc * P:(qc + 1) * P],
                    v_half[half][i][:, jj, :],
                    start=(i == 0), stop=(i == KT_TILES - 1),
                    skip_group_check=True,
                )
            flush_transpose()
            rec = rc_pool.tile([P, 1], f32, tag="rc", name="rec")
            nc.vector.reciprocal_approx_fast(rec[:], acc[:, 64:65])
            ctxn = cn_pool.tile([P, 64], f16, tag="cn", name="ctxn")
            nc.vector.tensor_scalar_mul(ctxn[:], acc[:, 0:64], rec[:, 0:1])
            pend_tr[0] = (bp, bqt, c, ctxn)

        def qk_slot(p, qt, kt):
            st = st_pool.tile([P, 2, NQ], f32, tag="st", name="st")
            for j in range(2):
                h = j * 64
                nc.tensor.matmul(
                    st[:, j, :],
                    KTs[p][h:h + 64, kt * P:(kt + 1) * P],
                    QTs[p][h:h + 64, qt * NQ:(qt + 1) * NQ],
                    start=True, stop=True,
                )
            pt = pt_pool.tile([P, 2, NQ], f16, tag="pt", name="pt")
            nc.scalar.activation(pt[:], st[:], AF.Exp, scale=0.03125)
            pt_map[(p, qt, kt)] = pt

        # ---------------- schedule ----------------
        # extra-work placement: dict (p, qt, kt) -> list of callables
        extra = {}

        def put(p, qt, kt, fn):
            extra.setdefault((p, qt, kt), []).append(fn)

        # pair 0 projections: K nt0 + Q nt0 upfront; rest dribbled in (0,0)
        alloc_pair(0)
        wqk_t[0] = (load_w(wk_pool, wkt_d, 0), load_w(wq_pool, wqt_d, 0))
        proj_nt(KTs[0], wqk_t[0][0], 0)
        proj_nt(QTs[0], wqk_t[0][1], 0)
        # remaining K-chunks of pair 0 needed at kt = 4*nt of every block
        for ntc in (1, 2, 3):
            put(0, 0, 2 * (ntc - 1) + 1,
                lambda ntc=ntc: proj_nt(KTs[0], wqk_t[0][0], ntc))
        # Q-chunks of pair 0: Qnt_c needed from block (0, c)
        for ntc in (1, 2, 3):
            put(0, ntc - 1, 9, lambda ntc=ntc: proj_nt(QTs[0], wqk_t[0][1], ntc))
        # v-projection half 0 (pairs 0,1): all 16 passes in block (0,0) --
        # the first AV chains (of block (0,0), run in block (0,1)) need
        # every v tile, so none may be emitted later.
        for mt in range(KT_TILES):
            put(0, 0, mt, lambda mt=mt: proj_v(mt, 0))
        # v-projection half 1 (pairs 2,3): spread over pair-1 blocks
        # (slots 10-15; chains own slots 1-8)
        for mt in range(KT_TILES):
            put(1, mt // 4, 10 + (mt % 4) + (mt % 4 > 1),
                lambda mt=mt: proj_v(mt, 1))

        # pair p+1 q/k projections: K chunks + Q nt0 during pair-p blocks,
        # Q nt1..3 just-in-time during pair p+1's first blocks (slots 9-12,
        # after the AV chains)
        for p in range(PAIRS - 1):
            pn = p + 1

            def loadw(pn=pn):
                alloc_pair(pn)
                wqk_t[pn] = (load_w(wk_pool, wkt_d, pn),
                             load_w(wq_pool, wqt_d, pn))
            put(p, 0, 0, loadw)
            for i in range(4):  # K chunks first
                put(p, 1 + i % 3, 9 + 2 * (i // 3),
                    lambda pn=pn, i=i: proj_nt(KTs[pn], wqk_t[pn][0], i))
            put(p, 2, 12, lambda pn=pn: proj_nt(QTs[pn], wqk_t[pn][1], 0))
            for i in (1, 2, 3):
                put(pn, i - 1, 12,
                    lambda pn=pn, i=i: proj_nt(QTs[pn], wqk_t[pn][1], i))

        # wot loads + o-proj chunks for pair p-1 during pair p's blocks
        put(0, 2, 1, lambda: load_wot(0))
        put(1, 0, 1, lambda: load_wot(1))
        put(2, 0, 1, lambda: load_wot(2))
        put(2, 2, 1, lambda: load_wot(3))
        for p in range(1, PAIRS):
            for qt in range(QT_TILES):
                put(p, qt, 10,
                    lambda p=p, qt=qt: oproj_chunk(p - 1, qt, range(4)))
                put(p, qt, 13,
                    lambda p=p, qt=qt: oproj_chunk(p - 1, qt, range(4, 8)))
        # pair 3's own o-proj: chunk qt right after its chains complete
        # (must flush the pending chain-7 transpose first)
        for qt in range(1, QT_TILES):
            put(3, qt, 15,
                lambda qt=qt: (flush_transpose(), oproj_chunk(3, qt - 1)))

        blocks = [(p, qt) for p in range(PAIRS) for qt in range(QT_TILES)]
        for bi, (p, qt) in enumerate(blocks):
            prev = blocks[bi - 1] if bi > 0 else None
            for kt in range(KT_TILES):
                qk_slot(p, qt, kt)
                # chains of the previous block at slots 1-8: finishing them
                # early gives the exp engine 8 slots of pt-ring slack at
                # every block boundary
                if prev is not None and 1 <= kt <= 8:
                    av_chain(prev[0], prev[1], kt - 1)
                for fn in extra.get((p, qt, kt), ()):
                    fn()
            if prev is not None:
                # release prev block's pt tiles from the map
                for kk in range(KT_TILES):
                    del pt_map[(prev[0], prev[1], kk)]
        # tail: chains of the last block + final o-proj chunk
        for c in range(8):
            av_chain(3, 3, c)
        flush_transpose()
        oproj_chunk(3, 3)

    nc.finalize()
    return nc


def _get_nc():
    global _NC_CACHE
    if _NC_CACHE is None:
        _NC_CACHE = _build_nc()
    return _NC_CACHE


def _make_in_maps(hidden_state, w_q, w_k, w_v, w_o):
    hidden_state = np.asarray(hidden_state, np.float32)
    w_q = np.asarray(w_q, np.float32)
    w_k = np.asarray(w_k, np.float32)
    w_v = np.asarray(w_v, np.float32)
    w_o = np.asarray(w_o, np.float32)

    ident = np.eye(P, dtype=np.float16)
    in_maps = []
    for core in range(NCORES):
        b, hh = core // 2, core % 2
        rows = slice(hh * 512, (hh + 1) * 512)
        xt = hidden_state[b].T.astype(np.float16).reshape(DC, P, S)
        # w[rows].T: [1024 d, 512 c] -> per-pair contiguous [4, 128, 8*128]
        wqt = (w_q[rows].T.reshape(DC, P, PAIRS, P).transpose(2, 1, 0, 3)
               .reshape(PAIRS, P, DC * P).astype(np.float16))
        wkt = (w_k[rows].T.reshape(DC, P, PAIRS, P).transpose(2, 1, 0, 3)
               .reshape(PAIRS, P, DC * P).astype(np.float16))
        wvt = w_v[rows].T.reshape(DC, P, 512).astype(np.float16)
        woth = np.ascontiguousarray(w_o[:, rows].T.reshape(PAIRS, P, D)
                                    ).astype(np.float16)
        in_maps.append({"xt": np.ascontiguousarray(xt),
                        "wqt": np.ascontiguousarray(wqt),
                        "wkt": np.ascontiguousarray(wkt),
                        "wvt": np.ascontiguousarray(wvt),
                        "woth": woth,
                        "ident": ident})
    return in_maps


def _assemble(results):
    out = np.empty((B, S, D), np.float32)
    for b in range(B):
        t = (results[2 * b]["outt"].reshape(D, S).astype(np.float32)
             + results[2 * b + 1]["outt"].reshape(D, S).astype(np.float32))
        out[b] = t.T
    return out


def run_spmd(hidden_state, w_q, w_k, w_v, w_o, **spmd_kwargs):
    """Run the kernel; returns (output, BassKernelResults)."""
    from concourse.bass_utils import run_bass_kernel_spmd

    nc = _get_nc()
    in_maps = _make_in_maps(hidden_state, w_q, w_k, w_v, w_o)
    res = run_bass_kernel_spmd(nc, in_maps, core_ids=list(range(NCORES)),
                               **spmd_kwargs)
    return _assemble(res.results), res


def kernel(hidden_state, attention_mask=None, w_q=None, w_k=None, w_v=None,
           w_o=None):
    out, _ = run_spmd(hidden_state, w_q, w_k, w_v, w_o)
    return out


# revision 24
# speedup vs baseline: 1.0307x; 1.0307x over previous
"""BertAttention Trainium2 kernel (8 NeuronCores, SPMD).

Sharding: core c handles batch b = c//2 and head-half hh = c%2 (8 of 16 heads).
Each core computes q/k/v projections for its 512 head-dims over its batch's
full sequence, per-head attention (no mask, scale 1/sqrt(1024)), and a partial
o-projection over its 512 context dims.  Host sums the two partials per batch.

v2 design (ACT-exp is the hard floor at ~285us/core; PE work reduced below it):
  QK  : unchanged (K=64 row-packed pairs, fp16, half-rate -- irreducible).
  exp : ACT over st [128,2,512] psum -> pt [128,2,512] f16, 256 instrs.
  AV  : SWAPPED dataflow -- stationary = P^T chunk [128k,128q] (from pt),
        moving = V_aug [128k,65] (64 v-dims + ones col). out = ctx [128q,65]
        psum, accumulated over 16 k-tiles per (head j, q-chunk qc).  Full
        128-wide array => ~2x fewer PE cycles than the ctx^T form.  The
        softmax denominator lands in psum COLUMN 64 -> per-partition DVE
        reciprocal + tensor_scalar_mul (no broadcast matmuls, no [64,2048]
        DVE multiplies).
  ctxT: PE transpose (identity matmul) of normalized ctx [128q,64] ->
        [64,128] written at column-position 64j, gpsimd copies psum->sbuf.
  oproj: as baseline (per-pair psum chunks, DVE add into out_sb), fp16 out.

Emission order keeps ACT continuously fed: per (pair p, qt) block, the 16
kt-slots emit QK+exp, and between them: AV chains of the PREVIOUS block
(1 chain per 2 slots), q/k projection bursts for pair p+1, v-projection
half-passes, and o-proj chunks for pair p-1.

PSUM (16KB/partition): st 2x[128,2,512]f32 (8KB) + acc 2x[128,65]f32 padded
to 2KB (4KB) + pp 2x2KB (proj/oproj/transpose, 4KB).
"""

import sys

sys.path.insert(0, "/opt/trn_rl_repo")

import numpy as np

B, S, D, H = 4, 2048, 1024, 16
HEAD = 64
NCORES = 8
P = 128
NQ = 512            # q free-tile width
KT_TILES = S // P   # 16 k tiles
QT_TILES = S // NQ  # 4 q tiles
DC = 8              # contraction chunks for projections (1024/128)
PAIRS = 4           # head pairs per core


_NC_CACHE = None


def _build_nc():
    import concourse.bass as bass  # noqa: F401
    import concourse.tile as tile
    from concourse import bacc, mybir

    f32 = mybir.dt.float32
    f16 = mybir.dt.float16
    AF = mybir.ActivationFunctionType

    nc = bacc.Bacc(None)
    xt_d = nc.declare_dram_parameter("xt", [DC, P, S], f16, isOutput=False)
    # per-pair contiguous weight images [P, DC*P]: one big DMA each (the
    # [DC,P,P] chunk layout produced 256B/partition packets, ~45us to land)
    wqt_d = nc.declare_dram_parameter("wqt", [PAIRS, P, DC * P], f16, isOutput=False)
    wkt_d = nc.declare_dram_parameter("wkt", [PAIRS, P, DC * P], f16, isOutput=False)
    wvt_d = nc.declare_dram_parameter("wvt", [DC, P, 512], f16, isOutput=False)
    woth_d = nc.declare_dram_parameter("woth", [PAIRS, P, D], f16, isOutput=False)
    ident_d = nc.declare_dram_parameter("ident", [P, P], f16, isOutput=False)
    out_d = nc.declare_dram_parameter("outt", [D // P, P, S], f16, isOutput=True)

    from contextlib import ExitStack

    with tile.TileContext(nc) as tc, ExitStack() as es:
        def pool(name, bufs, space="SBUF"):
            return es.enter_context(
                tc.tile_pool(name=name, bufs=bufs, space=space))

        xt_pool = pool("xt", 1)
        # 4 slots = all pairs' weights resident: pair p+1's weight DMA must
        # not wait on pair p's LAST (late-dribbled) proj burst, since pair
        # p+1's own bursts sit earlier in the in-order PE stream.
        wq_pool = pool("wq", 2)
        wk_pool = pool("wk", 2)
        wv_pool = pool("wv", 8)
        qt_pool = pool("qt", 2)
        kt_pool = pool("kt", 2)
        v_pool = pool("v", 1)
        pt_pool = pool("pt", 34)
        cn_pool = pool("cn", 4)
        rc_pool = pool("rc", 4)
        ctx_pool = pool("ctx", 1)
        wot_pool = pool("wot", 1)
        ost_pool = pool("ost", 1)
        on_pool = pool("on", 1)
        pp_pool = pool("pp", 2, "PSUM")
        st_pool = pool("st", 2, "PSUM")
        acc_pool = pool("acc", 2, "PSUM")

        # PE warmup during the initial DMA: keeps HAM at 8/8 so the
        # first projection matmuls run at 2.4 GHz
        wup = on_pool.tile([P, NQ], f16, tag="wup", name="wup")
        nc.vector.memset(wup[:], 0.125)
        wups = pp_pool.tile([P, NQ], f32, tag="pp", name="wups")
        for _ in range(42):
            nc.tensor.matmul(wups[:], wup[:, 0:P], wup[:],
                             start=True, stop=True)

        # load x^T chunks -- spread across 4 engine queues so the first
        # projections (which contract over ALL chunks) start ~4x sooner
        xt = []
        dma_engs = [nc.sync, nc.gpsimd, nc.scalar]
        for k in range(DC):
            t = xt_pool.tile([P, S], f16, tag=f"xt{k}", name=f"xt{k}")
            dma_engs[k % 3].dma_start(t[:], xt_d[k])
            xt.append(t)

        wv_t = []
        for k in range(DC):
            t = wv_pool.tile([P, NQ], f16, tag="wv", name="wv")
            dma_engs[(k + 1) % 3].dma_start(t[:], wvt_d[k])
            wv_t.append(t)

        # identity for PE transposes (after xt/wv in the queue)
        ident = on_pool.tile([P, P], f16, tag="id", name="ident")
        nc.gpsimd.dma_start(ident[:], ident_d[:, :])

        # V_aug tiles per k-tile: [128 keys, 4 heads, 65] (64 v-dims + ones)
        v_half = {0: [None] * KT_TILES, 1: [None] * KT_TILES}

        def proj_v(mt, half):
            """one N=256 projection pass filling v_half[half][mt]"""
            ps = pp_pool.tile([P, 256], f32, tag="pp", name="ppv")
            for k in range(DC):
                nc.tensor.matmul(
                    ps[:], xt[k][:, mt * P:(mt + 1) * P],
                    wv_t[k][:, half * 256:(half + 1) * 256],
                    start=(k == 0), stop=(k == DC - 1),
                )
            t = v_pool.tile([P, 4, 65], f16, tag=f"v{half}_{mt}",
                            name=f"v{half}_{mt}")
            nc.vector.memset(t[:], 1.0)
            v_half[half][mt] = t
            src = ps[:, :].rearrange("p (h d) -> p h d", h=4)
            nc.vector.tensor_copy(t[:, :, 0:64], src)

        def load_w(w_pool, w_dram, p):
            t = w_pool.tile([P, DC * P], f16, tag="w", name="w")
            nc.sync.dma_start(t[:], w_dram[p])
            return [t[:, k * P:(k + 1) * P] for k in range(DC)]

        def proj_nt(out, w_t, nt):
            ps = pp_pool.tile([P, NQ], f32, tag="pp", name="pp")
            for k in range(DC):
                nc.tensor.matmul(
                    ps[:], w_t[k][:], xt[k][:, nt * NQ:(nt + 1) * NQ],
                    start=(k == 0), stop=(k == DC - 1),
                )
            nc.vector.tensor_copy(out[:, nt * NQ:(nt + 1) * NQ], ps[:])

        # ---- per-pair state ----
        QTs, KTs, ctxTs = {}, {}, {}
        wot_t = {}
        wqk_t = {}
        out_sb = []
        for _ot in range(D // P):
            _t = ost_pool.tile([P, S], f16, tag=f"ou{_ot}", name=f"ou{_ot}")
            out_sb.append(_t)

        def load_wot(p):
            th = wot_pool.tile([P, D], f16, tag=f"woth{p}", name=f"woth{p}")
            nc.gpsimd.dma_start(th[:], woth_d[p])
            wot_t[p] = th

        def alloc_pair(p):
            KTs[p] = kt_pool.tile([P, S], f16, tag="t", name=f"kt{p}")
            QTs[p] = qt_pool.tile([P, S], f16, tag="t", name=f"qt{p}")
            ctxTs[p] = ctx_pool.tile([P, S], f16, tag=f"ctx{p}",
                                     name=f"ctx{p}")

        def oproj_chunk(p, qt, ots=None):
            for ot in (range(D // P) if ots is None else ots):
                ps = pp_pool.tile([P, NQ], f32, tag="pp", name="pp")
                nc.tensor.matmul(
                    ps[:], wot_t[p][:, ot * P:(ot + 1) * P],
                    ctxTs[p][:, qt * NQ:(qt + 1) * NQ],
                    start=True, stop=True,
                )
                dst = out_sb[ot][:, qt * NQ:(qt + 1) * NQ]
                if p == 0:
                    nc.vector.tensor_copy(dst, ps[:])
                else:
                    nc.vector.tensor_add(dst, dst, ps[:])
                if p == PAIRS - 1:
                    eng = nc.sync if ot % 2 == 0 else nc.gpsimd
                    eng.dma_start(out_d[ot][:, qt * NQ:(qt + 1) * NQ], dst)

        # pt tiles of the two most recent blocks
        pt_map = {}

        tp_box = [None]
        # deferred transpose: (bp, bqt, c, ctxn) emitted one chain later so
        # the PE never stalls on the DVE normalize (measured ~1.1us/chain)
        pend_tr = [None]

        def flush_transpose():
            if pend_tr[0] is None:
                return
            bp, bqt, c, ctxn = pend_tr[0]
            pend_tr[0] = None
            j, qc = c % 2, c // 2
            if j == 0:
                tp_box[0] = pp_pool.tile([P, P], f16, tag="pp", name="tp")
            tp = tp_box[0]
            nc.tensor.matmul(tp[64 * j:64 * (j + 1), :], ctxn[:], ident[:],
                             is_transpose=True)
            if j == 1:
                # gpsimd cannot read PSUM; DVE does the psum->sbuf hop
                nc.vector.tensor_copy(
                    ctxTs[bp][:, bqt * NQ + qc * P: bqt * NQ + (qc + 1) * P],
                    tp[:],
                )

        def av_chain(bp, bqt, c):
            """AV chain c (j = c%2, qc = c//2) of block (bp, bqt):
            ctx[q,d] accumulation + normalize; transpose deferred."""
            j, qc = c % 2, c // 2
            half = bp // 2
            jj = (2 * bp + j) % 4
            acc = acc_pool.tile([P, 65], f32, tag="acc", name="acc",
                                padded_shape=[P, 512])
            for i in range(KT_TILES):
                nc.tensor.matmul(
                    acc[:],
                    pt_map[(bp, bqt, i)][:, j, qc * P:(qc + 1) * P],
                    v_half[half][i][:, jj, :],
                    start=(i == 0), stop=(i == KT_TILES - 1),
                    skip_group_check=True,
                )
            flush_transpose()
            rec = rc_pool.tile([P, 1], f32, tag="rc", name="rec")
            nc.vector.reciprocal_approx_fast(rec[:], acc[:, 64:65])
            ctxn = cn_pool.tile([P, 64], f16, tag="cn", name="ctxn")
            nc.vector.tensor_scalar_mul(ctxn[:], acc[:, 0:64], rec[:, 0:1])
            pend_tr[0] = (bp, bqt, c, ctxn)

        def qk_slot(p, qt, kt):
            st = st_pool.tile([P, 2, NQ], f32, tag="st", name="st")
            for j in range(2):
                h = j * 64
                nc.tensor.matmul(
                    st[:, j, :],
                    KTs[p][h:h + 64, kt * P:(kt + 1) * P],
                    QTs[p][h:h + 64, qt * NQ:(qt + 1) * NQ],
                    start=True, stop=True,
                )
            pt = pt_pool.tile([P, 2, NQ], f16, tag="pt", name="pt")
            nc.scalar.activation(pt[:], st[:], AF.Exp, scale=0.03125)
            pt_map[(p, qt, kt)] = pt

        # ---------------- schedule ----------------
        # extra-work placement: dict (p, qt, kt) -> list of callables
        extra = {}

        def put(p, qt, kt, fn):
            extra.setdefault((p, qt, kt), []).append(fn)

        # pair 0 projections: K nt0 + Q nt0 upfront; rest dribbled in (0,0)
        alloc_pair(0)
        wqk_t[0] = (load_w(wk_pool, wkt_d, 0), load_w(wq_pool, wqt_d, 0))
        proj_nt(KTs[0], wqk_t[0][0], 0)
        proj_nt(QTs[0], wqk_t[0][1], 0)
        # remaining K-chunks of pair 0 needed at kt = 4*nt of every block
        for ntc in (1, 2, 3):
            put(0, 0, 4 * (ntc - 1) + 1,
                lambda ntc=ntc: proj_nt(KTs[0], wqk_t[0][0], ntc))
        # Q-chunks of pair 0: Qnt_c needed from block (0, c)
        for ntc in (1, 2, 3):
            put(0, ntc - 1, 6, lambda ntc=ntc: proj_nt(QTs[0], wqk_t[0][1], ntc))
        # v-projection half 0 (pairs 0,1): all 16 passes in block (0,0) --
        # the first AV chains (of block (0,0), run in block (0,1)) need
        # every v tile, so none may be emitted later.
        for mt in range(KT_TILES):
            put(0, 0, mt, lambda mt=mt: proj_v(mt, 0))
        # v-projection half 1 (pairs 2,3): spread over pair-1 blocks
        for mt in range(KT_TILES):
            put(1, mt // 4, (mt % 4) * 4 + 2, lambda mt=mt: proj_v(mt, 1))

        # pair p+1 q/k projections: 8 bursts spread over pair-p blocks
        for p in range(PAIRS - 1):
            pn = p + 1

            def loadw(pn=pn):
                alloc_pair(pn)
                wqk_t[pn] = (load_w(wk_pool, wkt_d, pn),
                             load_w(wq_pool, wqt_d, pn))
            put(p, 0, 0, loadw)
            for i in range(4):  # K chunks first
                put(p, 1 + i % 3, 3 + 2 * (i // 3),
                    lambda pn=pn, i=i: proj_nt(KTs[pn], wqk_t[pn][0], i))
            for i in range(4):
                put(p, 1 + i % 3, 9 + 2 * (i // 3),
                    lambda pn=pn, i=i: proj_nt(QTs[pn], wqk_t[pn][1], i))

        # wot loads + o-proj chunks for pair p-1 during pair p's blocks
        put(0, 2, 1, lambda: load_wot(0))
        put(1, 0, 1, lambda: load_wot(1))
        put(2, 0, 1, lambda: load_wot(2))
        put(2, 2, 1, lambda: load_wot(3))
        for p in range(1, PAIRS):
            for qt in range(QT_TILES):
                put(p, qt, 10,
                    lambda p=p, qt=qt: oproj_chunk(p - 1, qt, range(4)))
                put(p, qt, 13,
                    lambda p=p, qt=qt: oproj_chunk(p - 1, qt, range(4, 8)))
        # pair 3's own o-proj: chunk qt right after its chains complete
        # (must flush the pending chain-7 transpose first)
        for qt in range(1, QT_TILES):
            put(3, qt, 15,
                lambda qt=qt: (flush_transpose(), oproj_chunk(3, qt - 1)))

        blocks = [(p, qt) for p in range(PAIRS) for qt in range(QT_TILES)]
        for bi, (p, qt) in enumerate(blocks):
            prev = blocks[bi - 1] if bi > 0 else None
            for kt in range(KT_TILES):
                qk_slot(p, qt, kt)
                if prev is not None and kt % 2 == 1:
                    av_chain(prev[0], prev[1], kt // 2)
                for fn in extra.get((p, qt, kt), ()):
                    fn()
            if prev is not None:
                # release prev block's pt tiles from the map
                for kk in range(KT_TILES):
                    del pt_map[(prev[0], prev[1], kk)]
        # tail: chains of the last block + final o-proj chunk
        for c in range(8):
            av_chain(3, 3, c)
        flush_transpose()
        oproj_chunk(3, 3)

    nc.finalize()
    return nc


def _get_nc():
    global _NC_CACHE
    if _NC_CACHE is None:
        _NC_CACHE = _build_nc()
    return _NC_CACHE


def _make_in_maps(hidden_state, w_q, w_k, w_v, w_o):
    hidden_state = np.asarray(hidden_state, np.float32)
    w_q = np.asarray(w_q, np.float32)
    w_k = np.asarray(w_k, np.float32)
    w_v = np.asarray(w_v, np.float32)
    w_o = np.asarray(w_o, np.float32)

    ident = np.eye(P, dtype=np.float16)
    in_maps = []
    for core in range(NCORES):
        b, hh = core // 2, core % 2
        rows = slice(hh * 512, (hh + 1) * 512)
        xt = hidden_state[b].T.astype(np.float16).reshape(DC, P, S)
        # w[rows].T: [1024 d, 512 c] -> per-pair contiguous [4, 128, 8*128]
        wqt = (w_q[rows].T.reshape(DC, P, PAIRS, P).transpose(2, 1, 0, 3)
               .reshape(PAIRS, P, DC * P).astype(np.float16))
        wkt = (w_k[rows].T.reshape(DC, P, PAIRS, P).transpose(2, 1, 0, 3)
               .reshape(PAIRS, P, DC * P).astype(np.float16))
        wvt = w_v[rows].T.reshape(DC, P, 512).astype(np.float16)
        woth = np.ascontiguousarray(w_o[:, rows].T.reshape(PAIRS, P, D)
                                    ).astype(np.float16)
        in_maps.append({"xt": np.ascontiguousarray(xt),
                        "wqt": np.ascontiguousarray(wqt),
                        "wkt": np.ascontiguousarray(wkt),
                        "wvt": np.ascontiguousarray(wvt),
                        "woth": woth,
                        "ident": ident})
    return in_maps


def _assemble(results):
    out = np.empty((B, S, D), np.float32)
    for b in range(B):
        t = (results[2 * b]["outt"].reshape(D, S).astype(np.float32)
             + results[2 * b + 1]["outt"].reshape(D, S).astype(np.float32))
        out[b] = t.T
    return out


def run_spmd(hidden_state, w_q, w_k, w_v, w_o, **spmd_kwargs):
    """Run the kernel; returns (output, BassKernelResults)."""
    from concourse.bass_utils import run_bass_kernel_spmd

    nc = _get_nc()
    in_maps = _make_in_maps(hidden_state, w_q, w_k, w_v, w_o)
    res = run_bass_kernel_spmd(nc, in_maps, core_ids=list(range(NCORES)),
                               **spmd_kwargs)
    return _assemble(res.results), res


def kernel(hidden_state, attention_mask=None, w_q=None, w_k=None, w_v=None,
           w_o=None):
    out, _ = run_spmd(hidden_state, w_q, w_k, w_v, w_o)
    return out
